# revision 32
# baseline (speedup 1.0000x reference)
"""Trainium2 Bass kernel for the GNO (Galerkin-type linear attention) model.

Reference computation per batch element b (N=4096 tokens, d=64):
    h = x @ lift_w + lift_b
    for each of 4 layers:
        q = h@q_w+q_b ; k = h@k_w+k_b ; v = h@v_w+v_b
        kern     = (q @ k^T) / sqrt(d)          # [N, N], no softmax!
        integral = (kern @ v) / N               # [N, d]
        h        = gelu(h@blk_w+blk_b + integral)
    out = h @ proj_w + proj_b

Because there is no softmax, (q k^T) v == q (k^T v), and with the Gram
matrix G = H_aug H_aug^T (H_aug = [h^T; 1], a ones row folding biases):
    ktv  = k^T v = Wk_aug^T G Wv_aug
    wh + integral = H_aug^T W_upd,
    W_upd = blk_w_aug + A_l (G Wv_aug),  A_l = s q_w_aug Wk_aug^T  (host)
so each layer is: transpose pass T_c = H_chunk^T (via identity matmuls,
which also transposes the ones row into a ones column), Gram
accumulation G += T_c^T T_c, a tiny weight chain, then one [65,64]
combined update matmul + gelu.  This halves PSUM->SBUF evacuation
traffic vs. materializing k and v (2080 vs 4096 columns per layer),
which otherwise paces the tensor engine.

All matmul operands are bf16 (fp32 matmuls are split by the compiler
into 2 half-rate passes = 4x cost); PSUM accumulation stays fp32.
Verified numerically: bf16 pipeline via the G route lands at ~6e-3 rel
err vs the fp32 reference (tolerance 2e-2).

Schedule notes: phase-1 of layer l+1 (transpose + Gram) is software-
pipelined into layer l's update/gelu groups so the tensor engine follows
the scalar engine's gelu cadence instead of idling; the final projection
is likewise interleaved into layer 3's gelu tail.  Layer 0 skips the
transpose pass entirely: G_0 = lift_aug^T (Xa Xa^T) lift_aug, with the
tiny [4,4] input Gram accumulated from a token-major copy of x using
4-column weight loads, overlapped with the lift itself.

Sharding: batch is 2 -> data-parallel on 2 NeuronCores, one batch element
per core, everything resident in SBUF.  Sequence-sharding wider would need
a per-layer AllReduce whose ~10us latency floor exceeds the whole
per-layer compute, so it loses.
"""

import os
import sys

for _p in ("/opt/trn_rl_repo", "/root/.axon_site/_ro/trn_rl_repo"):
    if os.path.isdir(_p) and _p not in sys.path:
        sys.path.append(_p)

import numpy as np

N = 4096          # tokens per batch element (64*64)
D = 64            # hidden
DA = D + 1        # hidden + ones row
L = 4             # layers
B = 2             # batch / cores used
SCALE = (1.0 / np.sqrt(np.float32(D))) / np.float32(N)

# packed-weights column offsets: [lift | Wv | AlT | blkw | proj | ident | hv0]
_OFF_LIFT = 0
_OFF_WV = _OFF_LIFT + DA
_OFF_ALT = _OFF_WV + L * D
_OFF_BLKW = _OFF_ALT + L * DA
_OFF_PROJ = _OFF_BLKW + L * D
_OFF_IDENT = _OFF_PROJ + 1
_OFF_HV0 = _OFF_IDENT + DA
_WPACK = _OFF_HV0 + D

_CACHE = {}


def _build_nc():
    """Build + compile the per-core Bass program (identical on both cores)."""
    import concourse.bass as bass
    import concourse.tile as tile
    from concourse import bacc, mybir

    f32 = mybir.dt.float32
    bf16 = mybir.dt.bfloat16
    ts = bass.ts
    GELU = mybir.ActivationFunctionType.Gelu
    COPY = mybir.ActivationFunctionType.Copy

    nc = bacc.Bacc("TRN2", target_bir_lowering=False, debug=False, num_devices=B)

    xt_d = nc.dram_tensor("xt", [4, N], bf16, kind="ExternalInput")
    # x in token-major layout: xtt[p, 4c+j] = x_aug[token 128c+p, j]
    xtt_d = nc.dram_tensor("xtt", [128, 128], bf16, kind="ExternalInput")
    wp_d = nc.dram_tensor("wpack", [DA, _WPACK], bf16, kind="ExternalInput")
    # y is produced token-transposed: y_token(128*q + p) = y_d[p, q]
    y_d = nc.dram_tensor("y", [128, N // 128], f32, kind="ExternalOutput")

    PS = bass.MemorySpace.PSUM

    with tile.TileContext(nc) as tc:
        with (
            tc.tile_pool(name="consts", bufs=1) as consts,
            tc.tile_pool(name="hbuf", bufs=1) as hbuf,
            tc.tile_pool(name="tsb", bufs=3) as tsb,
            tc.tile_pool(name="small", bufs=2) as small,
            tc.tile_pool(name="ps_t", bufs=2, space=PS) as ps_t,
            tc.tile_pool(name="ps_sm", bufs=2, space=PS) as ps_sm,
            tc.tile_pool(name="ps_up", bufs=2, space=PS) as ps_up,
        ):
            # ---- load everything into SBUF (parallel DMA queues) ---------
            wp = consts.tile([DA, _WPACK], bf16, tag="wp")
            nc.sync.dma_start(wp[:], wp_d.ap())
            xtt = consts.tile([128, 128], bf16, tag="xtt")
            nc.scalar.dma_start(xtt[:], xtt_d.ap())
            xt = consts.tile([4, N], bf16, tag="xt")
            nc.scalar.dma_start(xt[:], xt_d.ap())

            liftw = wp[0:4, _OFF_LIFT : _OFF_LIFT + DA]
            wv = wp[:, _OFF_WV : _OFF_WV + L * D]
            alt = wp[:, _OFF_ALT : _OFF_ALT + L * DA]
            blkw = wp[:, _OFF_BLKW : _OFF_BLKW + L * D]
            projw = wp[:, _OFF_PROJ : _OFF_PROJ + 1]
            ident = wp[:, _OFF_IDENT : _OFF_IDENT + DA]
            hv0 = wp[0:4, _OFF_HV0 : _OFF_HV0 + D]

            # two ping-pong H_aug buffers, [65, 4096] each
            H0 = hbuf.tile([DA, N], bf16, tag="h0")
            H1 = hbuf.tile([DA, N], bf16, tag="h1")
            # H1's ones row never gets written by the layer loop; seed it
            # from the ones row of x^T_aug.  H0's comes from the lift matmul.
            nc.sync.dma_start(H1[D : D + 1, :], xt_d.ap()[3:4, :])

            # ---- ACT table preload (overlaps input DMAs) -----------------
            scratch = small.tile([1, 1], bf16, tag="scr")
            nc.scalar.activation(scratch[:], wp[0:1, 0:1], GELU)

            # ---- lift: H0 = lift_aug^T @ xt  ([65, 4096]) ----------------
            # 8 single-bank groups; copies alternate DVE/ACT so the psum
            # rotation never stalls the tensor engine.
            def lift_group(g):
                ps = ps_t.tile([128, 512], f32, tag="t")
                nc.tensor.matmul(ps[0:DA, :], liftw[:], xt[:, ts(g, 512)],
                                 start=True, stop=True)
                if g % 2 == 0:
                    nc.vector.tensor_copy(H0[:, ts(g, 512)], ps[0:DA, :])
                else:
                    nc.scalar.activation(H0[:, ts(g, 512)], ps[0:DA, :], COPY)

            # ---- per-layer phase-1: transpose pass + Gram accumulation ---
            # T group g: 4 chunks -> psum [128, 4*65]; one copy; 4 G matmuls
            def make_phase1(cur):
                g_ps = ps_sm.tile([DA, DA], f32, tag="sm")
                t_sbs = [None] * 8

                def t_group(g):
                    t_ps = ps_t.tile([128, 512], f32, tag="t")
                    for k in range(4):
                        t = 4 * g + k
                        # T_chunk [128, 65] = H_chunk^T (ones row -> col 64)
                        nc.tensor.matmul(t_ps[:, k * DA : (k + 1) * DA],
                                         cur[:, ts(t, 128)], ident[:],
                                         start=True, stop=True)
                    t_sb = tsb.tile([128, 4 * DA], bf16, tag="tsb")
                    nc.vector.tensor_copy(t_sb[:], t_ps[:, 0 : 4 * DA])
                    t_sbs[g] = t_sb

                def g_group(g):
                    t_sb = t_sbs[g]
                    for k in range(4):
                        first = (g == 0 and k == 0)
                        last = (g == 7 and k == 3)
                        # G += T_chunk^T @ T_chunk (fp32 accumulation)
                        nc.tensor.matmul(g_ps[:],
                                         t_sb[:, k * DA : (k + 1) * DA],
                                         t_sb[:, k * DA : (k + 1) * DA],
                                         start=first, stop=last)

                return t_group, g_group, g_ps

            # ---- layer 0 Gram shortcut: G_0 = lift_aug^T (Xa Xa^T) lift_aug
            # so no transpose pass over H0 is needed at all.  Gx = Xa Xa^T
            # is accumulated from the token-major x layout with tiny
            # 4-column weight loads, concurrently with the lift itself.
            gx_ps = ps_sm.tile([DA, DA], f32, tag="sm")
            for c in range(32):
                nc.tensor.matmul(gx_ps[0:4, 0:4], xtt[:, 4 * c : 4 * c + 4],
                                 xtt[:, 4 * c : 4 * c + 4],
                                 start=(c == 0), stop=(c == 31))
            for g in range(8):
                lift_group(g)

            yt_ps = None

            # ---- layers --------------------------------------------------
            for l in range(L):
                cur = H0 if l % 2 == 0 else H1
                nxt = H1 if l % 2 == 0 else H0

                # weight chain: W_upd = blkw_l + A_l @ (G @ Wv_l)
                if l == 0:
                    # m1 = G_0 Wv_0 = lift_aug^T @ (Gx @ (lift_aug Wv_aug_0))
                    gx_sb = small.tile([4, 4], bf16, tag="gxsb")
                    nc.scalar.activation(gx_sb[:], gx_ps[0:4, 0:4], COPY)
                    tmp_ps = ps_sm.tile([DA, DA], f32, tag="sm")
                    nc.tensor.matmul(tmp_ps[0:4, 0:D], gx_sb[:], hv0[:],
                                     start=True, stop=True)
                    tmp_sb = small.tile([4, D], bf16, tag="tmp")
                    nc.scalar.activation(tmp_sb[:], tmp_ps[0:4, 0:D], COPY)
                    m1_ps = ps_sm.tile([DA, DA], f32, tag="sm")
                    nc.tensor.matmul(m1_ps[:, 0:D], liftw[:], tmp_sb[:],
                                     start=True, stop=True)
                else:
                    # small chain copies ride the (idle) scalar engine so
                    # they never queue behind big DVE tile copies
                    g_sb = small.tile([DA, DA], bf16, tag="gsb")
                    nc.scalar.activation(g_sb[:], g_ps[:], COPY)
                    m1_ps = ps_sm.tile([DA, DA], f32, tag="sm")
                    nc.tensor.matmul(m1_ps[:, 0:D], g_sb[:],
                                     wv[:, l * D : (l + 1) * D],
                                     start=True, stop=True)
                m1_sb = small.tile([DA, D], bf16, tag="m1")
                nc.scalar.activation(m1_sb[:], m1_ps[:, 0:D], COPY)
                weff_ps = ps_sm.tile([DA, DA], f32, tag="sm")
                nc.tensor.matmul(weff_ps[:, 0:D],
                                 alt[:, l * DA : (l + 1) * DA], m1_sb[:],
                                 start=True, stop=False)
                # += blkw via PE (identity stationary) instead of a DVE add
                nc.tensor.matmul(weff_ps[:, 0:D], ident[:],
                                 blkw[:, l * D : (l + 1) * D],
                                 start=False, stop=True)
                wupd_sb = small.tile([DA, D], bf16, tag="wupd")
                nc.scalar.activation(wupd_sb[:], weff_ps[:, 0:D], COPY)

                if l + 1 < L:
                    t_g, g_g, g_ps = make_phase1(nxt)
                else:
                    yt_full = ps_sm.tile([128, DA], f32, tag="sm")
                    yt_ps = yt_full[:, 0 : N // 128]

                # h' = gelu(H_aug^T @ W_upd); phase-1 of layer l+1 (or the
                # projection, for the last layer) interleaves with the gelu
                # groups.  The update matmuls of group c+1 are emitted BEFORE
                # the transpose groups that wait on gelu(c), so the in-order
                # tensor engine never parks on the gelu semaphore while
                # independent update work is available.
                def upd_group(c):
                    ps = ps_up.tile([DA, 1024], f32, tag="up")
                    for i in range(2):
                        nc.tensor.matmul(
                            ps[0:D, ts(i, 512)], wupd_sb[:],
                            cur[:, 1024 * c + 512 * i : 1024 * c + 512 * (i + 1)],
                            start=True, stop=True)
                    return ps

                up_ps_c = upd_group(0)
                for c in range(4):
                    nc.scalar.activation(nxt[0:D, ts(c, 1024)], up_ps_c[0:D, :],
                                         GELU)
                    if c < 3:
                        up_ps_c = upd_group(c + 1)
                    if l + 1 < L:
                        t_g(2 * c)
                        if c > 0:
                            g_g(2 * c - 2)
                        t_g(2 * c + 1)
                        if c > 0:
                            g_g(2 * c - 1)
                    else:
                        # proj: yT[p, q] = y_token(128*q + p), chunk matmuls
                        for q in range(8 * c, 8 * c + 8):
                            nc.tensor.matmul(yt_ps[:, q : q + 1],
                                             nxt[:, ts(q, 128)], projw[:],
                                             start=True, stop=True)
                if l + 1 < L:
                    g_g(6)
                    g_g(7)

            out_sb = consts.tile([128, N // 128], f32, tag="out")
            nc.vector.tensor_copy(out_sb[:], yt_ps[:])
            nc.sync.dma_start(y_d.ap(), out_sb[:])

    nc.compile()
    return nc


def _prep_inputs(x, lift_w, lift_b, blk_w, blk_b, q_w, q_b, k_w, k_b, v_w,
                 v_b, proj_w, proj_b):
    """Host-side weight packing (tiny [64,64] reshuffles, negligible cost)."""
    from ml_dtypes import bfloat16

    f = lambda a: np.asarray(a, dtype=np.float32)
    bf = lambda a: np.ascontiguousarray(np.asarray(a, np.float32),
                                        dtype=np.float32).astype(bfloat16)
    x = f(x)
    lift_w, lift_b = f(lift_w), f(lift_b)
    blk_w, blk_b = f(blk_w), f(blk_b)
    q_w, q_b, k_w, k_b, v_w, v_b = f(q_w), f(q_b), f(k_w), f(k_b), f(v_w), f(v_b)
    proj_w, proj_b = f(proj_w), f(proj_b)

    wpack = np.zeros((DA, _WPACK), np.float32)
    # lift_aug [4, 65]: col 64 makes the lift matmul emit H0's ones row
    wpack[:3, _OFF_LIFT : _OFF_LIFT + D] = lift_w
    wpack[3, _OFF_LIFT : _OFF_LIFT + D] = lift_b
    wpack[3, _OFF_LIFT + D] = 1.0
    for l in range(L):
        wk_aug = np.vstack([k_w[l], k_b[l][None]])                # [65, 64]
        wv_aug = np.vstack([v_w[l], v_b[l][None]])                # [65, 64]
        q_aug = np.vstack([q_w[l], q_b[l][None]]) * SCALE         # [65, 64]
        a_l = q_aug @ wk_aug.T                                    # [65, 65]
        wpack[:, _OFF_WV + l * D : _OFF_WV + (l + 1) * D] = wv_aug
        wpack[:, _OFF_ALT + l * DA : _OFF_ALT + (l + 1) * DA] = a_l.T
        wpack[:, _OFF_BLKW + l * D : _OFF_BLKW + (l + 1) * D] = \
            np.vstack([blk_w[l], blk_b[l][None]])
    wpack[:, _OFF_PROJ] = np.concatenate([proj_w[:, 0], proj_b])
    wpack[:, _OFF_IDENT : _OFF_IDENT + DA] = np.eye(DA, dtype=np.float32)
    # hv0 = lift_aug @ Wv_aug_0 for the layer-0 Gram shortcut
    lift_aug = wpack[:4, _OFF_LIFT : _OFF_LIFT + DA]              # [4, 65]
    wv0_aug = np.vstack([v_w[0], v_b[0][None]])                   # [65, 64]
    wpack[:4, _OFF_HV0 : _OFF_HV0 + D] = lift_aug @ wv0_aug

    in_maps = []
    for b in range(B):
        xt = np.concatenate([x[b].reshape(N, 3).T,
                             np.ones((1, N), np.float32)], axis=0)
        xa = np.concatenate([x[b].reshape(N, 3),
                             np.ones((N, 1), np.float32)], axis=1)  # [N, 4]
        xtt = np.ascontiguousarray(
            xa.reshape(32, 128, 4).transpose(1, 0, 2).reshape(128, 128))
        in_maps.append({"xt": bf(xt), "xtt": bf(xtt), "wpack": bf(wpack)})
    return in_maps, x.shape


def _unpack_y(y_np):
    """[128, 32] token-transposed fp32 -> flat [4096] token order."""
    return np.ascontiguousarray(np.asarray(y_np, np.float32).T).reshape(N)


def _get_runner():
    """Compile once, return a fn(in_maps) -> list[{name: np.ndarray}]."""
    if "runner" in _CACHE:
        return _CACHE["runner"]

    import jax
    from jax.sharding import Mesh, PartitionSpec
    try:
        from jax.experimental.shard_map import shard_map
    except ImportError:  # newer jax
        from jax.sharding import shard_map
    from concourse import mybir
    from concourse.bass2jax import (_bass_exec_p, install_neuronx_cc_hook,
                                    partition_id_tensor)

    nc = _build_nc()
    install_neuronx_cc_hook()

    partition_name = (nc.partition_id_tensor.name
                      if nc.partition_id_tensor else None)
    in_names, out_names, out_avals, zero_outs = [], [], [], []
    for alloc in nc.m.functions[0].allocations:
        if not isinstance(alloc, mybir.MemoryLocationSet):
            continue
        name = alloc.memorylocations[0].name
        if alloc.kind == "ExternalInput":
            if name != partition_name:
                in_names.append(name)
        elif alloc.kind == "ExternalOutput":
            shape = tuple(alloc.tensor_shape)
            dtype = mybir.dt.np(alloc.dtype)
            out_names.append(name)
            out_avals.append(jax.core.ShapedArray(shape, dtype))
            zero_outs.append(np.zeros(shape, dtype))
    n_params = len(in_names)
    n_outs = len(out_avals)
    all_in_names = in_names + out_names + ([partition_name] if partition_name else [])
    donate = tuple(range(n_params, n_params + n_outs))

    def _body(*args):
        operands = list(args)
        if partition_name is not None:
            operands.append(partition_id_tensor())
        return tuple(_bass_exec_p.bind(
            *operands, out_avals=tuple(out_avals), in_names=tuple(all_in_names),
            out_names=tuple(out_names), lowering_input_output_aliases=(),
            sim_require_finite=True, sim_require_nnan=True, nc=nc))

    devices = jax.devices()[:B]
    mesh = Mesh(np.asarray(devices), ("core",))
    sharded = jax.jit(
        shard_map(_body, mesh=mesh,
                  in_specs=(PartitionSpec("core"),) * (n_params + n_outs),
                  out_specs=(PartitionSpec("core"),) * n_outs,
                  check_rep=False),
        donate_argnums=donate, keep_unused=True)

    def run(in_maps):
        per_core = [[np.asarray(m[name]) for name in in_names] for m in in_maps]
        concat_in = [np.concatenate([per_core[c][i] for c in range(B)], axis=0)
                     for i in range(n_params)]
        big_zeros = [np.concatenate([z] * B, axis=0) for z in zero_outs]
        outs = jax.block_until_ready(sharded(*concat_in, *big_zeros))
        results = []
        for c in range(B):
            r = {}
            for i, name in enumerate(out_names):
                rows = out_avals[i].shape[0]
                r[name] = np.asarray(outs[i][c * rows : (c + 1) * rows])
            results.append(r)
        return results

    _CACHE["runner"] = run
    return run


def kernel(**inputs) -> np.ndarray:
    in_maps, x_shape = _prep_inputs(**inputs)
    run = _get_runner()
    results = run(in_maps)
    out = np.stack([_unpack_y(results[b]["y"]).reshape(x_shape[1], x_shape[2], 1)
                    for b in range(B)])
    return out.astype(np.float32)


# revision 34
# speedup vs baseline: 1.0287x; 1.0287x over previous
"""Trainium2 Bass kernel for the GNO (Galerkin-type linear attention) model.

Reference computation per batch element b (N=4096 tokens, d=64):
    h = x @ lift_w + lift_b
    for each of 4 layers:
        q = h@q_w+q_b ; k = h@k_w+k_b ; v = h@v_w+v_b
        kern     = (q @ k^T) / sqrt(d)          # [N, N], no softmax!
        integral = (kern @ v) / N               # [N, d]
        h        = gelu(h@blk_w+blk_b + integral)
    out = h @ proj_w + proj_b

Because there is no softmax, (q k^T) v == q (k^T v), and with the Gram
matrix G = H_aug H_aug^T (H_aug = [h^T; 1], a ones row folding biases):
    ktv  = k^T v = Wk_aug^T G Wv_aug
    wh + integral = H_aug^T W_upd,
    W_upd = blk_w_aug + A_l (G Wv_aug),  A_l = s q_w_aug Wk_aug^T  (host)
so each layer is: transpose pass T_c = H_chunk^T (via identity matmuls,
which also transposes the ones row into a ones column), Gram
accumulation G += T_c^T T_c, a tiny weight chain, then one [65,64]
combined update matmul + gelu.  This halves PSUM->SBUF evacuation
traffic vs. materializing k and v (2080 vs 4096 columns per layer),
which otherwise paces the tensor engine.

All matmul operands are bf16 (fp32 matmuls are split by the compiler
into 2 half-rate passes = 4x cost); PSUM accumulation stays fp32.
Verified numerically: bf16 pipeline via the G route lands at ~6e-3 rel
err vs the fp32 reference (tolerance 2e-2).

Schedule notes: phase-1 of layer l+1 (transpose + Gram) is software-
pipelined into layer l's update/gelu groups so the tensor engine follows
the scalar engine's gelu cadence instead of idling; the final projection
is likewise interleaved into layer 3's gelu tail.  Layer 0 skips the
transpose pass entirely: G_0 = lift_aug^T (Xa Xa^T) lift_aug, with the
tiny [4,4] input Gram accumulated from a token-major copy of x using
4-column weight loads, overlapped with the lift itself.

Sharding: batch is 2 -> data-parallel on 2 NeuronCores, one batch element
per core, everything resident in SBUF.  Sequence-sharding wider would need
a per-layer AllReduce whose ~10us latency floor exceeds the whole
per-layer compute, so it loses.
"""

import os
import sys

for _p in ("/opt/trn_rl_repo", "/root/.axon_site/_ro/trn_rl_repo"):
    if os.path.isdir(_p) and _p not in sys.path:
        sys.path.append(_p)

import numpy as np

N = 4096          # tokens per batch element (64*64)
D = 64            # hidden
DA = D + 1        # hidden + ones row
L = 4             # layers
B = 2             # batch / cores used
SCALE = (1.0 / np.sqrt(np.float32(D))) / np.float32(N)

# packed-weights column offsets: [lift | Wv | AlT | blkw | proj | ident | hv0]
_OFF_LIFT = 0
_OFF_WV = _OFF_LIFT + DA
_OFF_ALT = _OFF_WV + L * D
_OFF_BLKW = _OFF_ALT + L * DA
_OFF_PROJ = _OFF_BLKW + L * D
_OFF_IDENT = _OFF_PROJ + 1
_OFF_HV0 = _OFF_IDENT + DA
_WPACK = _OFF_HV0 + D

_CACHE = {}


def _build_nc():
    """Build + compile the per-core Bass program (identical on both cores)."""
    import concourse.bass as bass
    import concourse.tile as tile
    from concourse import bacc, mybir

    f32 = mybir.dt.float32
    bf16 = mybir.dt.bfloat16
    ts = bass.ts
    GELU = mybir.ActivationFunctionType.Gelu
    COPY = mybir.ActivationFunctionType.Copy

    nc = bacc.Bacc("TRN2", target_bir_lowering=False, debug=False, num_devices=B)

    xt_d = nc.dram_tensor("xt", [4, N], bf16, kind="ExternalInput")
    # x in token-major layout: xtt[p, 4c+j] = x_aug[token 128c+p, j]
    xtt_d = nc.dram_tensor("xtt", [128, 128], bf16, kind="ExternalInput")
    wp_d = nc.dram_tensor("wpack", [DA, _WPACK], bf16, kind="ExternalInput")
    # y is produced token-transposed: y_token(128*q + p) = y_d[p, q]
    y_d = nc.dram_tensor("y", [128, N // 128], f32, kind="ExternalOutput")

    PS = bass.MemorySpace.PSUM

    with tile.TileContext(nc) as tc:
        with (
            tc.tile_pool(name="consts", bufs=1) as consts,
            tc.tile_pool(name="hbuf", bufs=1) as hbuf,
            tc.tile_pool(name="tsb", bufs=3) as tsb,
            tc.tile_pool(name="small", bufs=2) as small,
            tc.tile_pool(name="ps_t", bufs=3, space=PS) as ps_t,
            tc.tile_pool(name="ps_sm", bufs=1, space=PS) as ps_sm,
            tc.tile_pool(name="ps_up", bufs=2, space=PS) as ps_up,
        ):
            # ---- load everything into SBUF (parallel DMA queues) ---------
            wp = consts.tile([DA, _WPACK], bf16, tag="wp")
            nc.sync.dma_start(wp[:], wp_d.ap())
            xtt = consts.tile([128, 128], bf16, tag="xtt")
            nc.scalar.dma_start(xtt[:], xtt_d.ap())
            xt = consts.tile([4, N], bf16, tag="xt")
            nc.scalar.dma_start(xt[:], xt_d.ap())

            liftw = wp[0:4, _OFF_LIFT : _OFF_LIFT + DA]
            wv = wp[:, _OFF_WV : _OFF_WV + L * D]
            alt = wp[:, _OFF_ALT : _OFF_ALT + L * DA]
            blkw = wp[:, _OFF_BLKW : _OFF_BLKW + L * D]
            projw = wp[:, _OFF_PROJ : _OFF_PROJ + 1]
            ident = wp[:, _OFF_IDENT : _OFF_IDENT + DA]
            hv0 = wp[0:4, _OFF_HV0 : _OFF_HV0 + D]

            # two ping-pong H_aug buffers, [65, 4096] each
            H0 = hbuf.tile([DA, N], bf16, tag="h0")
            H1 = hbuf.tile([DA, N], bf16, tag="h1")
            # H1's ones row never gets written by the layer loop; seed it
            # from the ones row of x^T_aug.  H0's comes from the lift matmul.
            nc.sync.dma_start(H1[D : D + 1, :], xt_d.ap()[3:4, :])

            # ---- ACT table preload (overlaps input DMAs) -----------------
            scratch = small.tile([1, 1], bf16, tag="scr")
            nc.scalar.activation(scratch[:], wp[0:1, 0:1], GELU)

            # ---- lift: H0 = lift_aug^T @ xt  ([65, 4096]) ----------------
            # 8 single-bank groups; copies alternate DVE/ACT so the psum
            # rotation never stalls the tensor engine.
            def lift_group(g):
                ps = ps_t.tile([128, 512], f32, tag="t")
                nc.tensor.matmul(ps[0:DA, :], liftw[:], xt[:, ts(g, 512)],
                                 start=True, stop=True)
                if g % 2 == 0:
                    nc.vector.tensor_copy(H0[:, ts(g, 512)], ps[0:DA, :])
                else:
                    nc.scalar.activation(H0[:, ts(g, 512)], ps[0:DA, :], COPY)

            # ---- per-layer phase-1: transpose pass + Gram accumulation ---
            # T group g: 4 chunks -> psum [128, 4*65]; one copy; 4 G matmuls
            def make_phase1(cur):
                g_ps = ps_sm.tile([DA, DA], f32, tag="sm")
                t_sbs = [None] * 8

                def t_group(g):
                    t_ps = ps_t.tile([128, 512], f32, tag="t")
                    for k in range(4):
                        t = 4 * g + k
                        # T_chunk [128, 65] = H_chunk^T (ones row -> col 64)
                        nc.tensor.matmul(t_ps[:, k * DA : (k + 1) * DA],
                                         cur[:, ts(t, 128)], ident[:],
                                         start=True, stop=True)
                    t_sb = tsb.tile([128, 4 * DA], bf16, tag="tsb")
                    nc.vector.tensor_copy(t_sb[:], t_ps[:, 0 : 4 * DA])
                    t_sbs[g] = t_sb

                def g_group(g):
                    t_sb = t_sbs[g]
                    for k in range(4):
                        first = (g == 0 and k == 0)
                        last = (g == 7 and k == 3)
                        # G += T_chunk^T @ T_chunk (fp32 accumulation)
                        nc.tensor.matmul(g_ps[:],
                                         t_sb[:, k * DA : (k + 1) * DA],
                                         t_sb[:, k * DA : (k + 1) * DA],
                                         start=first, stop=last)

                return t_group, g_group, g_ps

            # ---- layer 0 Gram shortcut: G_0 = lift_aug^T (Xa Xa^T) lift_aug
            # so no transpose pass over H0 is needed at all.  Gx = Xa Xa^T
            # is accumulated from the token-major x layout with tiny
            # 4-column weight loads, concurrently with the lift itself.
            gx_ps = ps_sm.tile([DA, DA], f32, tag="sm")
            for c in range(32):
                nc.tensor.matmul(gx_ps[0:4, 0:4], xtt[:, 4 * c : 4 * c + 4],
                                 xtt[:, 4 * c : 4 * c + 4],
                                 start=(c == 0), stop=(c == 31))
            for g in range(8):
                lift_group(g)

            yt_ps = None

            # ---- layers --------------------------------------------------
            for l in range(L):
                cur = H0 if l % 2 == 0 else H1
                nxt = H1 if l % 2 == 0 else H0

                # weight chain: W_upd = blkw_l + A_l @ (G @ Wv_l)
                if l == 0:
                    # m1 = G_0 Wv_0 = lift_aug^T @ (Gx @ (lift_aug Wv_aug_0))
                    gx_sb = small.tile([4, 4], bf16, tag="gxsb")
                    nc.scalar.activation(gx_sb[:], gx_ps[0:4, 0:4], COPY)
                    tmp_ps = ps_sm.tile([DA, DA], f32, tag="sm")
                    nc.tensor.matmul(tmp_ps[0:4, 0:D], gx_sb[:], hv0[:],
                                     start=True, stop=True)
                    tmp_sb = small.tile([4, D], bf16, tag="tmp")
                    nc.scalar.activation(tmp_sb[:], tmp_ps[0:4, 0:D], COPY)
                    m1_ps = ps_sm.tile([DA, DA], f32, tag="sm")
                    nc.tensor.matmul(m1_ps[:, 0:D], liftw[:], tmp_sb[:],
                                     start=True, stop=True)
                else:
                    # small chain copies ride the (idle) scalar engine so
                    # they never queue behind big DVE tile copies
                    g_sb = small.tile([DA, DA], bf16, tag="gsb")
                    nc.scalar.activation(g_sb[:], g_ps[:], COPY)
                    m1_ps = ps_sm.tile([DA, DA], f32, tag="sm")
                    nc.tensor.matmul(m1_ps[:, 0:D], g_sb[:],
                                     wv[:, l * D : (l + 1) * D],
                                     start=True, stop=True)
                m1_sb = small.tile([DA, D], bf16, tag="m1")
                nc.scalar.activation(m1_sb[:], m1_ps[:, 0:D], COPY)
                weff_ps = ps_sm.tile([DA, DA], f32, tag="sm")
                nc.tensor.matmul(weff_ps[:, 0:D],
                                 alt[:, l * DA : (l + 1) * DA], m1_sb[:],
                                 start=True, stop=False)
                # += blkw via PE (identity stationary) instead of a DVE add
                nc.tensor.matmul(weff_ps[:, 0:D], ident[:],
                                 blkw[:, l * D : (l + 1) * D],
                                 start=False, stop=True)
                wupd_sb = small.tile([DA, D], bf16, tag="wupd")
                nc.scalar.activation(wupd_sb[:], weff_ps[:, 0:D], COPY)

                if l + 1 < L:
                    t_g, g_g, g_ps = make_phase1(nxt)
                else:
                    yt_full = ps_sm.tile([128, DA], f32, tag="sm")
                    yt_ps = yt_full[:, 0 : N // 128]

                # h' = gelu(H_aug^T @ W_upd); phase-1 of layer l+1 (or the
                # projection, for the last layer) interleaves with the gelu
                # groups so the tensor engine stays busy.
                for c in range(4):
                    up_ps = ps_up.tile([DA, 1024], f32, tag="up")
                    for i in range(2):
                        nc.tensor.matmul(
                            up_ps[0:D, ts(i, 512)], wupd_sb[:],
                            cur[:, 1024 * c + 512 * i : 1024 * c + 512 * (i + 1)],
                            start=True, stop=True)
                    nc.scalar.activation(nxt[0:D, ts(c, 1024)], up_ps[0:D, :],
                                         GELU)
                    if l + 1 < L:
                        t_g(2 * c)
                        if c > 0:
                            g_g(2 * c - 2)
                        t_g(2 * c + 1)
                        if c > 0:
                            g_g(2 * c - 1)
                    else:
                        # proj: yT[p, q] = y_token(128*q + p), chunk matmuls
                        for q in range(8 * c, 8 * c + 8):
                            nc.tensor.matmul(yt_ps[:, q : q + 1],
                                             nxt[:, ts(q, 128)], projw[:],
                                             start=True, stop=True)
                if l + 1 < L:
                    g_g(6)
                    g_g(7)

            out_sb = consts.tile([128, N // 128], f32, tag="out")
            nc.vector.tensor_copy(out_sb[:], yt_ps[:])
            nc.sync.dma_start(y_d.ap(), out_sb[:])

    nc.compile()
    return nc


def _prep_inputs(x, lift_w, lift_b, blk_w, blk_b, q_w, q_b, k_w, k_b, v_w,
                 v_b, proj_w, proj_b):
    """Host-side weight packing (tiny [64,64] reshuffles, negligible cost)."""
    from ml_dtypes import bfloat16

    f = lambda a: np.asarray(a, dtype=np.float32)
    bf = lambda a: np.ascontiguousarray(np.asarray(a, np.float32),
                                        dtype=np.float32).astype(bfloat16)
    x = f(x)
    lift_w, lift_b = f(lift_w), f(lift_b)
    blk_w, blk_b = f(blk_w), f(blk_b)
    q_w, q_b, k_w, k_b, v_w, v_b = f(q_w), f(q_b), f(k_w), f(k_b), f(v_w), f(v_b)
    proj_w, proj_b = f(proj_w), f(proj_b)

    wpack = np.zeros((DA, _WPACK), np.float32)
    # lift_aug [4, 65]: col 64 makes the lift matmul emit H0's ones row
    wpack[:3, _OFF_LIFT : _OFF_LIFT + D] = lift_w
    wpack[3, _OFF_LIFT : _OFF_LIFT + D] = lift_b
    wpack[3, _OFF_LIFT + D] = 1.0
    for l in range(L):
        wk_aug = np.vstack([k_w[l], k_b[l][None]])                # [65, 64]
        wv_aug = np.vstack([v_w[l], v_b[l][None]])                # [65, 64]
        q_aug = np.vstack([q_w[l], q_b[l][None]]) * SCALE         # [65, 64]
        a_l = q_aug @ wk_aug.T                                    # [65, 65]
        wpack[:, _OFF_WV + l * D : _OFF_WV + (l + 1) * D] = wv_aug
        wpack[:, _OFF_ALT + l * DA : _OFF_ALT + (l + 1) * DA] = a_l.T
        wpack[:, _OFF_BLKW + l * D : _OFF_BLKW + (l + 1) * D] = \
            np.vstack([blk_w[l], blk_b[l][None]])
    wpack[:, _OFF_PROJ] = np.concatenate([proj_w[:, 0], proj_b])
    wpack[:, _OFF_IDENT : _OFF_IDENT + DA] = np.eye(DA, dtype=np.float32)
    # hv0 = lift_aug @ Wv_aug_0 for the layer-0 Gram shortcut
    lift_aug = wpack[:4, _OFF_LIFT : _OFF_LIFT + DA]              # [4, 65]
    wv0_aug = np.vstack([v_w[0], v_b[0][None]])                   # [65, 64]
    wpack[:4, _OFF_HV0 : _OFF_HV0 + D] = lift_aug @ wv0_aug

    in_maps = []
    for b in range(B):
        xt = np.concatenate([x[b].reshape(N, 3).T,
                             np.ones((1, N), np.float32)], axis=0)
        xa = np.concatenate([x[b].reshape(N, 3),
                             np.ones((N, 1), np.float32)], axis=1)  # [N, 4]
        xtt = np.ascontiguousarray(
            xa.reshape(32, 128, 4).transpose(1, 0, 2).reshape(128, 128))
        in_maps.append({"xt": bf(xt), "xtt": bf(xtt), "wpack": bf(wpack)})
    return in_maps, x.shape


def _unpack_y(y_np):
    """[128, 32] token-transposed fp32 -> flat [4096] token order."""
    return np.ascontiguousarray(np.asarray(y_np, np.float32).T).reshape(N)


def _get_runner():
    """Compile once, return a fn(in_maps) -> list[{name: np.ndarray}]."""
    if "runner" in _CACHE:
        return _CACHE["runner"]

    import jax
    from jax.sharding import Mesh, PartitionSpec
    try:
        from jax.experimental.shard_map import shard_map
    except ImportError:  # newer jax
        from jax.sharding import shard_map
    from concourse import mybir
    from concourse.bass2jax import (_bass_exec_p, install_neuronx_cc_hook,
                                    partition_id_tensor)

    nc = _build_nc()
    install_neuronx_cc_hook()

    partition_name = (nc.partition_id_tensor.name
                      if nc.partition_id_tensor else None)
    in_names, out_names, out_avals, zero_outs = [], [], [], []
    for alloc in nc.m.functions[0].allocations:
        if not isinstance(alloc, mybir.MemoryLocationSet):
            continue
        name = alloc.memorylocations[0].name
        if alloc.kind == "ExternalInput":
            if name != partition_name:
                in_names.append(name)
        elif alloc.kind == "ExternalOutput":
            shape = tuple(alloc.tensor_shape)
            dtype = mybir.dt.np(alloc.dtype)
            out_names.append(name)
            out_avals.append(jax.core.ShapedArray(shape, dtype))
            zero_outs.append(np.zeros(shape, dtype))
    n_params = len(in_names)
    n_outs = len(out_avals)
    all_in_names = in_names + out_names + ([partition_name] if partition_name else [])
    donate = tuple(range(n_params, n_params + n_outs))

    def _body(*args):
        operands = list(args)
        if partition_name is not None:
            operands.append(partition_id_tensor())
        return tuple(_bass_exec_p.bind(
            *operands, out_avals=tuple(out_avals), in_names=tuple(all_in_names),
            out_names=tuple(out_names), lowering_input_output_aliases=(),
            sim_require_finite=True, sim_require_nnan=True, nc=nc))

    devices = jax.devices()[:B]
    mesh = Mesh(np.asarray(devices), ("core",))
    sharded = jax.jit(
        shard_map(_body, mesh=mesh,
                  in_specs=(PartitionSpec("core"),) * (n_params + n_outs),
                  out_specs=(PartitionSpec("core"),) * n_outs,
                  check_rep=False),
        donate_argnums=donate, keep_unused=True)

    def run(in_maps):
        per_core = [[np.asarray(m[name]) for name in in_names] for m in in_maps]
        concat_in = [np.concatenate([per_core[c][i] for c in range(B)], axis=0)
                     for i in range(n_params)]
        big_zeros = [np.concatenate([z] * B, axis=0) for z in zero_outs]
        outs = jax.block_until_ready(sharded(*concat_in, *big_zeros))
        results = []
        for c in range(B):
            r = {}
            for i, name in enumerate(out_names):
                rows = out_avals[i].shape[0]
                r[name] = np.asarray(outs[i][c * rows : (c + 1) * rows])
            results.append(r)
        return results

    _CACHE["runner"] = run
    return run


def kernel(**inputs) -> np.ndarray:
    in_maps, x_shape = _prep_inputs(**inputs)
    run = _get_runner()
    results = run(in_maps)
    out = np.stack([_unpack_y(results[b]["y"]).reshape(x_shape[1], x_shape[2], 1)
                    for b in range(B)])
    return out.astype(np.float32)
